# revision 1
# baseline (speedup 1.0000x reference)
import numpy as np
from contextlib import ExitStack

import concourse.mybir as mybir
import concourse.bass as bass
import concourse.tile as tile
from concourse.bass_utils import run_bass_kernel_spmd

# Problem: nn_Predictor (moe_routing). L=6 streams, B=16384, D=512, NC=3992, 4 experts.
# Sharding: pure data parallel over B across 8 cores; weights replicated.
L, B, D, NCLS, NE = 6, 16384, 512, 3992, 4
NCORES = 8
BS = B // NCORES            # 2048 tokens per core
TT = 512                    # token tile
NTILES = BS // TT           # 4
NSUB = TT // 128            # 4 token subtiles per tile
KC = 24                     # 128-wide K chunks of flat (6*512/128)
NCH = (NCLS + 511) // 512   # 8 output column chunks (last = 408)

F32 = mybir.dt.float32
F32R = mybir.dt.float32r


def _r(ap):
    return ap.bitcast(F32R)


def _build():
    nc = bass.Bass("TRN2")

    fusion = nc.dram_tensor("fusion", [L, BS, D], F32, kind="ExternalInput")
    masksT = nc.dram_tensor("masksT", [NE, BS], F32, kind="ExternalInput")
    w1_in = [
        nc.dram_tensor("w1_0", [1536, 512], F32, kind="ExternalInput"),
        nc.dram_tensor("w1_1", [1536, 512], F32, kind="ExternalInput"),
        nc.dram_tensor("w1_2", [3072, 512], F32, kind="ExternalInput"),
        nc.dram_tensor("w1_3", [3072, 512], F32, kind="ExternalInput"),
    ]
    b1all = nc.dram_tensor("b1all", [NE * 512], F32, kind="ExternalInput")
    w2all = nc.dram_tensor("w2all", [NE, 512, 512], F32, kind="ExternalInput")
    b2s = nc.dram_tensor("b2s", [1, NE * 512], F32, kind="ExternalInput")
    dw1 = nc.dram_tensor("dw1", [512, 512], F32, kind="ExternalInput")
    db1 = nc.dram_tensor("db1", [512], F32, kind="ExternalInput")
    dw2 = nc.dram_tensor("dw2", [512, NCLS], F32, kind="ExternalInput")
    db2 = nc.dram_tensor("db2", [1, NCLS], F32, kind="ExternalInput")
    identD = nc.dram_tensor("ident128", [128, 128], F32, kind="ExternalInput")
    out = nc.dram_tensor("out", [BS, NCLS], F32, kind="ExternalOutput")

    # M-tile table for the W1 stage: (expert, flatT chunk range)
    # e0 eats front (chunks 0..11), e1 back (12..23), e2/e3 all 24.
    # e3's input scaling (a on front, b on back) is folded into w1_3 on host.
    w1_mtiles = []
    for e, (klo, nk) in enumerate([(0, 12), (12, 12), (0, 24), (0, 24)]):
        for mloc in range(4):
            w1_mtiles.append((e, mloc, klo, nk))

    with tile.TileContext(nc) as tc, ExitStack() as ctx:
        singles = ctx.enter_context(tc.tile_pool(name="singles", bufs=1))
        natP = ctx.enter_context(tc.tile_pool(name="natP", bufs=3))
        flatP = ctx.enter_context(tc.tile_pool(name="flatP", bufs=KC + 1))
        w1P = ctx.enter_context(tc.tile_pool(name="w1P", bufs=2))
        htP = ctx.enter_context(tc.tile_pool(name="htP", bufs=3))
        mbP = ctx.enter_context(tc.tile_pool(name="mbP", bufs=5))
        selP = ctx.enter_context(tc.tile_pool(name="selP", bufs=4))
        sigP = ctx.enter_context(tc.tile_pool(name="sigP", bufs=5))
        dw2P = ctx.enter_context(tc.tile_pool(name="dw2P", bufs=2))
        outP = ctx.enter_context(tc.tile_pool(name="outP", bufs=2))
        db2bcP = ctx.enter_context(tc.tile_pool(name="db2bcP", bufs=2))

        tposePs = ctx.enter_context(tc.tile_pool(name="tposePs", bufs=1, space="PSUM"))
        w1Ps = ctx.enter_context(tc.tile_pool(name="w1Ps", bufs=1, space="PSUM"))
        w2Ps = ctx.enter_context(tc.tile_pool(name="w2Ps", bufs=4, space="PSUM"))
        d2Ps = ctx.enter_context(tc.tile_pool(name="d2Ps", bufs=2, space="PSUM"))

        # identity via DMA (not Pool) so transposes carry a single coalesced
        # DMA-semaphore wait: walrus fits only one sync wait on the LW struct.
        ident = singles.tile([128, 128], F32R)
        nc.sync.dma_start(out=ident, in_=_r(identD[:, :]))

        # biases
        b1sb = singles.tile([128, 16], F32)     # [:, mi] = b1 of W1-stage M-tile mi
        nc.sync.dma_start(
            out=b1sb, in_=bass.AP(tensor=b1all, offset=0, ap=[[1, 128], [128, 16]])
        )
        b2sb = singles.tile([1, NE * 512], F32)
        nc.sync.dma_start(out=_r(b2sb), in_=_r(b2s[:, :]))
        db1sb = singles.tile([128, 4], F32)
        nc.sync.dma_start(
            out=db1sb, in_=bass.AP(tensor=db1, offset=0, ap=[[1, 128], [128, 4]])
        )

        # resident weights: W2 (lhsT layout) and dec_W1 (lhsT layout)
        w2sb = []
        for e in range(NE):
            w2e = singles.tile([128, 4 * 512], F32, name=f"w2sb{e}")
            nc.sync.dma_start(
                out=_r(w2e),
                in_=_r(bass.AP(
                    tensor=w2all,
                    offset=e * 512 * 512,
                    ap=[[512, 128], [128 * 512, 4], [1, 512]],
                )),
            )
            w2sb.append(w2e)
        dw1sb = singles.tile([128, 4 * 512], F32)
        nc.sync.dma_start(
            out=_r(dw1sb),
            in_=_r(bass.AP(tensor=dw1, offset=0, ap=[[512, 128], [128 * 512, 4], [1, 512]])),
        )

        for it in range(NTILES):
            t0 = it * TT

            # ---- stage A: load + transpose -> flatT chunks [128 feat, 512 tok]
            flatT = []
            for c in range(KC):
                l, off = c // 4, (c % 4) * 128
                natc = natP.tile([128, NSUB, 128], F32, name="natc")
                nc.sync.dma_start(
                    out=_r(natc),
                    in_=_r(bass.AP(
                        tensor=fusion,
                        offset=l * BS * D + t0 * D + off,
                        ap=[[D, 128], [128 * D, NSUB], [1, 128]],
                    )),
                )
                pT = tposePs.tile([128, TT], F32, name="pT")
                for s in range(NSUB):
                    nc.tensor.transpose(
                        _r(pT[:, s * 128 : (s + 1) * 128]), _r(natc[:, s, :]), _r(ident)
                    )
                fc = flatP.tile([128, TT], F32, name="fc")
                nc.any.tensor_copy(out=_r(fc), in_=pT)
                flatT.append(fc)

            # ---- broadcast one-hot expert masks [128, TT] per expert
            maskB = []
            for e in range(NE):
                mb = mbP.tile([128, TT], F32, name="mb")
                nc.sync.dma_start(
                    out=_r(mb),
                    in_=_r(bass.AP(
                        tensor=masksT, offset=e * BS + t0, ap=[[0, 128], [1, TT]]
                    )),
                )
                maskB.append(mb)

            # ---- stage B+C fused: W1 + bias + relu + mask, each ht chunk
            # immediately accumulated into the 4 selT psum banks via W2.
            w2ps = [w2Ps.tile([128, TT], F32, name="w2ps") for _ in range(4)]
            for mi, (e, mloc, klo, nk) in enumerate(w1_mtiles):
                ps = w1Ps.tile([128, TT], F32, name="w1ps")
                ki = 0
                for kb in range(0, nk, 12):
                    nb = min(12, nk - kb)
                    w1t = w1P.tile([128, nb * 128], F32, name="w1t")
                    nc.sync.dma_start(
                        out=_r(w1t),
                        in_=_r(bass.AP(
                            tensor=w1_in[e],
                            offset=(kb * 512 * 128) + mloc * 128,
                            ap=[[512, 128], [128 * 512, nb], [1, 128]],
                        )),
                    )
                    for kj in range(nb):
                        nc.tensor.matmul(
                            ps,
                            _r(w1t[:, kj * 128 : (kj + 1) * 128]),
                            _r(flatT[klo + ki]),
                            start=(ki == 0),
                            stop=(ki == nk - 1),
                        )
                        ki += 1
                h = htP.tile([128, TT], F32, name="h")
                nc.scalar.activation(
                    _r(h), ps, mybir.ActivationFunctionType.Relu,
                    bias=b1sb[:, mi : mi + 1], scale=1.0,
                )
                nc.vector.tensor_tensor(
                    out=_r(h), in0=h, in1=maskB[e], op=mybir.AluOpType.mult
                )
                for md in range(4):
                    nc.tensor.matmul(
                        w2ps[md],
                        _r(w2sb[e][:, mloc * 512 + md * 128 : mloc * 512 + md * 128 + 128]),
                        _r(h),
                        start=(mi == 0),
                        stop=False,
                    )

            # selected-expert W2 bias, then copy selT out of PSUM
            selT = []
            for md in range(4):
                for e in range(NE):
                    nc.tensor.matmul(
                        w2ps[md],
                        _r(b2sb[0:1, e * 512 + md * 128 : e * 512 + (md + 1) * 128]),
                        _r(maskB[e][0:1, :]),
                        start=False,
                        stop=(e == NE - 1),
                    )
                st = selP.tile([128, TT], F32, name="st")
                nc.any.tensor_copy(out=_r(st), in_=w2ps[md])
                selT.append(st)

            # ---- stage D: dec1 + sigmoid -> sigT [4][128 h2, TT]
            sigT = []
            for mh in range(4):
                ps = w1Ps.tile([128, TT], F32, name="w1ps")
                for kd in range(4):
                    nc.tensor.matmul(
                        ps,
                        _r(dw1sb[:, kd * 512 + mh * 128 : kd * 512 + mh * 128 + 128]),
                        _r(selT[kd]),
                        start=(kd == 0),
                        stop=(kd == 3),
                    )
                sg = sigP.tile([128, TT], F32, name="sg")
                nc.scalar.activation(
                    _r(sg), ps, mybir.ActivationFunctionType.Sigmoid,
                    bias=db1sb[:, mh : mh + 1], scale=1.0,
                )
                sigT.append(sg)

            # ---- stage E: dec2 (flip to natural) + bias -> out
            for n in range(NCH):
                nw = min(512, NCLS - n * 512)
                dwt = dw2P.tile([128, 4, nw], F32, name="dwt")
                nc.sync.dma_start(
                    out=_r(dwt),
                    in_=_r(bass.AP(
                        tensor=dw2,
                        offset=n * 512,
                        ap=[[NCLS, 128], [128 * NCLS, 4], [1, nw]],
                    )),
                )
                db2bc = db2bcP.tile([128, nw], F32, name="db2bc")
                nc.sync.dma_start(
                    out=db2bc,
                    in_=bass.AP(tensor=db2, offset=n * 512, ap=[[0, 128], [1, nw]]),
                )
                for s in range(NSUB):
                    ps = d2Ps.tile([128, 512], F32, name="d2ps")
                    for kh in range(4):
                        nc.tensor.matmul(
                            ps[:, :nw],
                            _r(sigT[kh][:, s * 128 : (s + 1) * 128]),
                            _r(dwt[:, kh, :]),
                            start=(kh == 0),
                            stop=(kh == 3),
                        )
                    ot = outP.tile([128, 512], F32, name="ot")
                    nc.vector.tensor_tensor(
                        out=ot[:, :nw], in0=ps[:, :nw], in1=db2bc,
                        op=mybir.AluOpType.add,
                    )
                    nc.sync.dma_start(
                        out=out[t0 + s * 128 : t0 + (s + 1) * 128, n * 512 : n * 512 + nw],
                        in_=ot[:, :nw],
                    )
    # walrus allows at most 1 sync wait per Matmult; split extras into
    # EventSemaphore instructions (same pass Bacc.compile runs)
    import bass_rust

    bass_rust.generate_event_semaphores(nc)
    return nc


_NC_CACHE = None


def _get_nc():
    global _NC_CACHE
    if _NC_CACHE is None:
        _NC_CACHE = _build()
    return _NC_CACHE


def _prep_inputs(inputs):
    f32 = np.float32
    x = np.asarray(inputs["fusion_hs"], f32)                      # [L, B, D]
    flat = np.transpose(x, (1, 0, 2)).reshape(B, L * D)

    logits = flat.astype(np.float64) @ np.asarray(inputs["gate_W"], f32).astype(
        np.float64
    ) + np.asarray(inputs["gate_b"], f32).astype(np.float64)
    am = np.argmax(logits, axis=1)
    masksT = np.zeros((NE, B), f32)
    masksT[am, np.arange(B)] = 1.0

    w1_3s = np.array(inputs["e3_W1"], f32, copy=True)
    w1_3s[: 3 * D] *= f32(np.asarray(inputs["e3_a"]).reshape(-1)[0])
    w1_3s[3 * D :] *= f32(np.asarray(inputs["e3_b"]).reshape(-1)[0])

    common = {
        "w1_0": np.ascontiguousarray(inputs["e0_W1"], f32),
        "w1_1": np.ascontiguousarray(inputs["e1_W1"], f32),
        "w1_2": np.ascontiguousarray(inputs["e2_W1"], f32),
        "w1_3": np.ascontiguousarray(w1_3s),
        "b1all": np.concatenate(
            [np.asarray(inputs[f"e{e}_b1"], f32) for e in range(NE)]
        ),
        "w2all": np.ascontiguousarray(
            np.stack([np.asarray(inputs[f"e{e}_W2"], f32) for e in range(NE)])
        ),
        "b2s": np.concatenate(
            [np.asarray(inputs[f"e{e}_b2"], f32) for e in range(NE)]
        ).reshape(1, NE * 512),
        "dw1": np.ascontiguousarray(inputs["dec_W1"], f32),
        "db1": np.ascontiguousarray(inputs["dec_b1"], f32),
        "dw2": np.ascontiguousarray(inputs["dec_W2"], f32),
        "db2": np.ascontiguousarray(
            np.asarray(inputs["dec_b2"], f32).reshape(1, NCLS)
        ),
        "ident128": np.eye(128, dtype=f32),
    }
    in_maps = []
    for c in range(NCORES):
        sl = slice(c * BS, (c + 1) * BS)
        m = dict(common)
        m["fusion"] = np.ascontiguousarray(x[:, sl, :])
        m["masksT"] = np.ascontiguousarray(masksT[:, sl])
        in_maps.append(m)
    return in_maps


def kernel(**inputs):
    nc = _get_nc()
    in_maps = _prep_inputs(inputs)
    res = run_bass_kernel_spmd(nc, in_maps, core_ids=list(range(NCORES)))
    return np.concatenate([res.results[c]["out"] for c in range(NCORES)], axis=0)



# revision 2
# speedup vs baseline: 2.6240x; 2.6240x over previous
import numpy as np
import ml_dtypes
from contextlib import ExitStack

import concourse.mybir as mybir
import concourse.bass as bass
import concourse.tile as tile
from concourse.bass_utils import run_bass_kernel_spmd

# nn_Predictor (moe_routing): L=6 streams, B=16384, D=512, NC=3992, 4 experts.
# Strategy: host computes the hard gate (argmax) and routes tokens to their
# expert; each core gets an equal, 128-padded share of every expert's tokens,
# pre-transposed ([feature, token]) and packed in bf16 so the device does no
# transposes and each token runs only its own expert's W1/W2. Decoder runs on
# the routed column space; host scatters rows back and adds dec_b2.
L, B, D, NCLS, NE = 6, 16384, 512, 3992, 4
NCORES = 8
PAD = 128                   # per-expert per-core column padding
TW = 512                    # column tile width
NCH = (NCLS + 511) // 512   # 8 output column chunks (last = 408)
STREAMS = [(0, 3), (3, 6), (0, 6), (0, 6)]
NK = [3 * D // 128, 3 * D // 128, 6 * D // 128, 6 * D // 128]  # 12,12,24,24

F32 = mybir.dt.float32
BF16 = mybir.dt.bfloat16
BF = ml_dtypes.bfloat16


def _build(C):
    """C: per-core padded column count per expert (multiples of PAD)."""
    nc = bass.Bass("TRN2")
    NTOT = sum(C)

    xin = [
        nc.dram_tensor(f"x{e}", [128, NK[e] * C[e]], BF16, kind="ExternalInput")
        for e in range(NE) if C[e]
    ]
    xmap = {e: xin[i] for i, e in enumerate([e for e in range(NE) if C[e]])}
    w1d = [
        nc.dram_tensor(f"w1_{e}", [128, NK[e] * 512], BF16, kind="ExternalInput")
        for e in range(NE)
    ]
    w2d = nc.dram_tensor("w2all", [128, NE * 2048], BF16, kind="ExternalInput")
    dw1d = nc.dram_tensor("dw1", [128, 2048], BF16, kind="ExternalInput")
    dw2d = nc.dram_tensor("dw2", [128, 4 * NCLS], BF16, kind="ExternalInput")
    b1d = nc.dram_tensor("b1p", [128, 16], F32, kind="ExternalInput")
    b2d = nc.dram_tensor("b2p", [128, 16], F32, kind="ExternalInput")
    db1d = nc.dram_tensor("db1p", [128, 4], F32, kind="ExternalInput")
    outD = nc.dram_tensor("out", [NTOT, NCLS], F32, kind="ExternalOutput")

    # column tiles: (expert, global col offset, in-expert offset, width)
    tiles = []
    off = 0
    for e in range(NE):
        for lo in range(0, C[e], TW):
            tiles.append((e, off + lo, lo, min(TW, C[e] - lo)))
        off += C[e]

    with tile.TileContext(nc) as tc, ExitStack() as ctx:
        singles = ctx.enter_context(tc.tile_pool(name="singles", bufs=1))
        xtP = ctx.enter_context(tc.tile_pool(name="xtP", bufs=2))
        hP = ctx.enter_context(tc.tile_pool(name="hP", bufs=3))
        selP = ctx.enter_context(tc.tile_pool(name="selP", bufs=2))
        sigP = ctx.enter_context(tc.tile_pool(name="sigP", bufs=6))
        outP = ctx.enter_context(tc.tile_pool(name="outP", bufs=4))

        hPs = ctx.enter_context(tc.tile_pool(name="hPs", bufs=2, space="PSUM"))
        w2Ps = ctx.enter_context(tc.tile_pool(name="w2Ps", bufs=4, space="PSUM"))
        d2Ps = ctx.enter_context(tc.tile_pool(name="d2Ps", bufs=2, space="PSUM"))

        # weights + biases via gpsimd SWDGE so the sync queue is free for the
        # per-tile x loads (w1[0]/w2/dw1 first: needed by tile 0's W1/W2/dec1;
        # dw2 before the later experts' W1 so tile 0's dec2 isn't starved).
        w1sb = [singles.tile([128, NK[e] * 512], BF16, name=f"w1sb{e}") for e in range(NE)]
        w2sb = singles.tile([128, NE * 2048], BF16)
        dw1sb = singles.tile([128, 2048], BF16)
        dwsb = singles.tile([128, 4, NCLS], BF16)
        b1sb = singles.tile([128, 16], F32)
        b2sb = singles.tile([128, 16], F32)
        db1sb = singles.tile([128, 4], F32)
        nc.gpsimd.dma_start(out=w1sb[0], in_=w1d[0][:, :])
        nc.gpsimd.dma_start(out=b1sb, in_=b1d[:, :])
        nc.gpsimd.dma_start(out=b2sb, in_=b2d[:, :])
        nc.gpsimd.dma_start(out=db1sb, in_=db1d[:, :])
        nc.gpsimd.dma_start(out=w2sb, in_=w2d[:, :])
        nc.gpsimd.dma_start(out=dw1sb, in_=dw1d[:, :])
        nc.gpsimd.dma_start(
            out=dwsb, in_=bass.AP(tensor=dw2d, offset=0, ap=[[4 * NCLS, 128], [NCLS, 4], [1, NCLS]])
        )
        for e in range(1, NE):
            nc.gpsimd.dma_start(out=w1sb[e], in_=w1d[e][:, :])

        for (e, goff, lo, wd) in tiles:
            nk = NK[e]
            xt = xtP.tile([128, nk, wd], BF16, name="xt")
            nc.sync.dma_start(
                out=xt,
                in_=bass.AP(tensor=xmap[e], offset=nk * lo,
                            ap=[[NK[e] * C[e], 128], [wd, nk], [1, wd]]),
            )

            # W1 (+relu +b1) fused into W2 accumulation over hidden chunks
            w2ps = [w2Ps.tile([128, wd], F32, name="w2ps") for _ in range(4)]
            for m in range(4):
                ps = hPs.tile([128, wd], F32, name="hps")
                for kc in range(nk):
                    nc.tensor.matmul(
                        ps,
                        w1sb[e][:, (kc * 4 + m) * 128:(kc * 4 + m + 1) * 128],
                        xt[:, kc, :],
                        start=(kc == 0),
                        stop=(kc == nk - 1),
                    )
                h = hP.tile([128, wd], BF16, name="h")
                nc.scalar.activation(
                    h, ps, mybir.ActivationFunctionType.Relu,
                    bias=b1sb[:, e * 4 + m:e * 4 + m + 1], scale=1.0,
                )
                for md in range(4):
                    nc.tensor.matmul(
                        w2ps[md],
                        w2sb[:, ((e * 4 + m) * 4 + md) * 128:((e * 4 + m) * 4 + md + 1) * 128],
                        h,
                        start=(m == 0),
                        stop=(m == 3),
                    )

            # sel = W2 out + b2, cast to bf16
            selt = selP.tile([128, 4, wd], BF16, name="selt")
            for md in range(4):
                nc.scalar.activation(
                    selt[:, md, :], w2ps[md], mybir.ActivationFunctionType.Identity,
                    bias=b2sb[:, e * 4 + md:e * 4 + md + 1], scale=1.0,
                )

            # dec1 + sigmoid
            sig4 = []
            for mh in range(4):
                ps = hPs.tile([128, wd], F32, name="hps")
                for kd in range(4):
                    nc.tensor.matmul(
                        ps,
                        dw1sb[:, (kd * 4 + mh) * 128:(kd * 4 + mh + 1) * 128],
                        selt[:, kd, :],
                        start=(kd == 0),
                        stop=(kd == 3),
                    )
                sg = sigP.tile([128, wd], BF16, name="sg")
                nc.scalar.activation(
                    sg, ps, mybir.ActivationFunctionType.Sigmoid,
                    bias=db1sb[:, mh:mh + 1], scale=1.0,
                )
                sig4.append(sg)

            # dec2: out[tok, cls] per 128-token subtile (db2 added on host)
            for s in range(wd // 128):
                for n in range(NCH):
                    nw = min(512, NCLS - n * 512)
                    ps2 = d2Ps.tile([128, 512], F32, name="d2ps")
                    for kh in range(4):
                        nc.tensor.matmul(
                            ps2[:, :nw],
                            sig4[kh][:, s * 128:(s + 1) * 128],
                            dwsb[:, kh, n * 512:n * 512 + nw],
                            start=(kh == 0),
                            stop=(kh == 3),
                        )
                    ot = outP.tile([128, 512], F32, name="ot")
                    nc.vector.tensor_copy(out=ot[:, :nw], in_=ps2[:, :nw])
                    nc.scalar.dma_start(
                        out=outD[goff + s * 128:goff + (s + 1) * 128, n * 512:n * 512 + nw],
                        in_=ot[:, :nw],
                    )

    import bass_rust

    bass_rust.generate_event_semaphores(nc)
    return nc


_NC_CACHE = {}


def _get_nc(C=None):
    if C is None:
        assert _NC_CACHE, "kernel not built yet"
        return next(iter(_NC_CACHE.values()))
    key = tuple(C)
    if key not in _NC_CACHE:
        _NC_CACHE[key] = _build(list(key))
    return _NC_CACHE[key]


def _lhsT_pack(w):
    # [K, M] -> [128, (K/128) * M], kc-major then m-chunk then m-inner
    K, M = w.shape
    nk, nm = K // 128, M // 128
    return np.ascontiguousarray(
        w.reshape(nk, 128, nm, 128).transpose(1, 0, 2, 3).reshape(128, nk * M).astype(BF)
    )


def _route(inputs):
    f32 = np.float32
    x = np.asarray(inputs["fusion_hs"], f32)
    flat = np.transpose(x, (1, 0, 2)).reshape(B, L * D)
    logits = flat.astype(np.float64) @ np.asarray(inputs["gate_W"], f32).astype(
        np.float64
    ) + np.asarray(inputs["gate_b"], f32).astype(np.float64)
    am = np.argmax(logits, axis=1)
    idx = [np.nonzero(am == e)[0] for e in range(NE)]
    percore = [[idx[e][c::NCORES] for c in range(NCORES)] for e in range(NE)]
    C = [
        int(np.ceil(max(len(percore[e][c]) for c in range(NCORES)) / PAD) * PAD)
        if len(idx[e]) else 0
        for e in range(NE)
    ]
    return x, percore, C


def _pack_x(x, idxc, e, Ce):
    l0, l1 = STREAMS[e]
    nl = l1 - l0
    K = nl * D
    nk = K // 128
    n = len(idxc)
    Xe = np.zeros((K, Ce), dtype=BF)
    if n:
        Xe[:, :n] = x[l0:l1, idxc, :].transpose(0, 2, 1).reshape(K, n).astype(BF)
    blocks = []
    for lo in range(0, Ce, TW):
        wd = min(TW, Ce - lo)
        blocks.append(
            Xe[:, lo:lo + wd].reshape(nk, 128, wd).transpose(1, 0, 2).reshape(128, nk * wd)
        )
    return np.ascontiguousarray(np.concatenate(blocks, axis=1))


def _prep_inputs(inputs):
    f32 = np.float32
    x, percore, C = _route(inputs)

    w13 = np.array(inputs["e3_W1"], f32, copy=True)
    w13[: 3 * D] *= f32(np.asarray(inputs["e3_a"]).reshape(-1)[0])
    w13[3 * D:] *= f32(np.asarray(inputs["e3_b"]).reshape(-1)[0])
    w1s = [np.asarray(inputs["e0_W1"], f32), np.asarray(inputs["e1_W1"], f32),
           np.asarray(inputs["e2_W1"], f32), w13]

    common = {f"w1_{e}": _lhsT_pack(w1s[e]) for e in range(NE)}
    common["w2all"] = np.concatenate(
        [_lhsT_pack(np.asarray(inputs[f"e{e}_W2"], f32)) for e in range(NE)], axis=1
    )
    common["dw1"] = _lhsT_pack(np.asarray(inputs["dec_W1"], f32))
    common["dw2"] = np.ascontiguousarray(
        np.asarray(inputs["dec_W2"], f32).reshape(4, 128, NCLS)
        .transpose(1, 0, 2).reshape(128, 4 * NCLS).astype(BF)
    )
    common["b1p"] = np.ascontiguousarray(
        np.concatenate([np.asarray(inputs[f"e{e}_b1"], f32) for e in range(NE)])
        .reshape(16, 128).T
    )
    common["b2p"] = np.ascontiguousarray(
        np.concatenate([np.asarray(inputs[f"e{e}_b2"], f32) for e in range(NE)])
        .reshape(16, 128).T
    )
    common["db1p"] = np.ascontiguousarray(
        np.asarray(inputs["dec_b1"], f32).reshape(4, 128).T
    )

    in_maps = []
    for c in range(NCORES):
        m = dict(common)
        for e in range(NE):
            if C[e]:
                m[f"x{e}"] = _pack_x(x, percore[e][c], e, C[e])
        in_maps.append(m)
    return in_maps, percore, C


def kernel(**inputs):
    in_maps, percore, C = _prep_inputs(inputs)
    nc = _get_nc(C)
    res = run_bass_kernel_spmd(nc, in_maps, core_ids=list(range(NCORES)))
    offs = np.concatenate([[0], np.cumsum(C)])
    out = np.empty((B, NCLS), np.float32)
    for c in range(NCORES):
        r = res.results[c]["out"]
        for e in range(NE):
            ic = percore[e][c]
            if len(ic):
                out[ic] = r[offs[e]:offs[e] + len(ic)]
    out += np.asarray(inputs["dec_b2"], np.float32).reshape(1, NCLS)
    return out


# revision 3
# speedup vs baseline: 2.9527x; 1.1252x over previous
import numpy as np
import ml_dtypes
from contextlib import ExitStack

import concourse.mybir as mybir
import concourse.bass as bass
import concourse.tile as tile
from concourse.bass_utils import run_bass_kernel_spmd

# nn_Predictor (moe_routing): L=6 streams, B=16384, D=512, NC=3992, 4 experts.
# Strategy: host computes the hard gate (argmax) and routes tokens to their
# expert; each core gets an equal, 128-padded share of every expert's tokens,
# pre-transposed ([feature, token]) and packed in bf16 so the device does no
# transposes and each token runs only its own expert's W1/W2. Decoder runs on
# the routed column space; host scatters rows back and adds dec_b2.
L, B, D, NCLS, NE = 6, 16384, 512, 3992, 4
NCORES = 8
PAD = 128                   # per-expert per-core column padding
TW = 512                    # column tile width
NCH = (NCLS + 511) // 512   # 8 output column chunks (last = 408)
STREAMS = [(0, 3), (3, 6), (0, 6), (0, 6)]
NK = [3 * D // 128, 3 * D // 128, 6 * D // 128, 6 * D // 128]  # 12,12,24,24

F32 = mybir.dt.float32
BF16 = mybir.dt.bfloat16
BF = ml_dtypes.bfloat16


def _build(C):
    """C: per-core padded column count per expert (multiples of PAD)."""
    nc = bass.Bass("TRN2")
    NTOT = sum(C)

    xin = [
        nc.dram_tensor(f"x{e}", [128, NK[e] * C[e]], BF16, kind="ExternalInput")
        for e in range(NE) if C[e]
    ]
    xmap = {e: xin[i] for i, e in enumerate([e for e in range(NE) if C[e]])}
    w1d = [
        nc.dram_tensor(f"w1_{e}", [128, NK[e] * 512], BF16, kind="ExternalInput")
        for e in range(NE)
    ]
    w2d = nc.dram_tensor("w2all", [128, NE * 2048], BF16, kind="ExternalInput")
    dw1d = nc.dram_tensor("dw1", [128, 2048], BF16, kind="ExternalInput")
    dw2d = nc.dram_tensor("dw2", [128, 4 * NCLS], BF16, kind="ExternalInput")
    b1d = nc.dram_tensor("b1p", [128, 16], F32, kind="ExternalInput")
    b2d = nc.dram_tensor("b2p", [128, 16], F32, kind="ExternalInput")
    db1d = nc.dram_tensor("db1p", [128, 4], F32, kind="ExternalInput")
    outD = nc.dram_tensor("out", [NTOT, NCLS], F32, kind="ExternalOutput")

    # column tiles: (expert, global col offset, in-expert offset, width)
    tiles = []
    off = 0
    for e in range(NE):
        for lo in range(0, C[e], TW):
            tiles.append((e, off + lo, lo, min(TW, C[e] - lo)))
        off += C[e]

    with tile.TileContext(nc) as tc, ExitStack() as ctx:
        singles = ctx.enter_context(tc.tile_pool(name="singles", bufs=1))
        xtP = ctx.enter_context(tc.tile_pool(name="xtP", bufs=2))
        hP = ctx.enter_context(tc.tile_pool(name="hP", bufs=3))
        selP = ctx.enter_context(tc.tile_pool(name="selP", bufs=2))
        sigP = ctx.enter_context(tc.tile_pool(name="sigP", bufs=6))
        outP = ctx.enter_context(tc.tile_pool(name="outP", bufs=4))

        hPs = ctx.enter_context(tc.tile_pool(name="hPs", bufs=2, space="PSUM"))
        w2Ps = ctx.enter_context(tc.tile_pool(name="w2Ps", bufs=4, space="PSUM"))
        d2Ps = ctx.enter_context(tc.tile_pool(name="d2Ps", bufs=2, space="PSUM"))

        # Load order matters for startup: the first tile's expert W1 rides the
        # sync queue ahead of the x tiles; everything else goes via gpsimd
        # SWDGE (biases first — tiny; dw2 split per class chunk so tile 0's
        # dec2 isn't starved; remaining experts' W1 last, in first-use order).
        w1sb = [singles.tile([128, NK[e] * 512], BF16, name=f"w1sb{e}") for e in range(NE)]
        w2sb = singles.tile([128, NE * 2048], BF16)
        dw1sb = singles.tile([128, 2048], BF16)
        dwsb = singles.tile([128, 4, NCLS], BF16)
        b1sb = singles.tile([128, 16], F32)
        b2sb = singles.tile([128, 16], F32)
        db1sb = singles.tile([128, 4], F32)
        e_first = tiles[0][0]
        nc.sync.dma_start(out=w1sb[e_first], in_=w1d[e_first][:, :])
        nc.gpsimd.dma_start(out=b1sb, in_=b1d[:, :])
        nc.gpsimd.dma_start(out=b2sb, in_=b2d[:, :])
        nc.gpsimd.dma_start(out=db1sb, in_=db1d[:, :])
        nc.gpsimd.dma_start(out=w2sb, in_=w2d[:, :])
        nc.gpsimd.dma_start(out=dw1sb, in_=dw1d[:, :])
        for n in range(NCH):
            nw = min(512, NCLS - n * 512)
            nc.gpsimd.dma_start(
                out=dwsb[:, :, n * 512:n * 512 + nw],
                in_=bass.AP(tensor=dw2d, offset=n * 512,
                            ap=[[4 * NCLS, 128], [NCLS, 4], [1, nw]]),
            )
        seen = {e_first}
        for (e, _, _, _) in tiles:
            if e not in seen:
                seen.add(e)
                nc.gpsimd.dma_start(out=w1sb[e], in_=w1d[e][:, :])

        def emit_w1w2(e, goff, lo, wd):
            nk = NK[e]
            xt = xtP.tile([128, nk, wd], BF16, name="xt")
            nc.sync.dma_start(
                out=xt,
                in_=bass.AP(tensor=xmap[e], offset=nk * lo,
                            ap=[[NK[e] * C[e], 128], [wd, nk], [1, wd]]),
            )
            # W1 + relu + b1, feeding W2 accumulation; W2(m) is emitted after
            # W1(m+1)'s chain so the relu never stalls the tensor engine.
            w2ps = [w2Ps.tile([128, wd], F32, name="w2ps") for _ in range(4)]
            hbuf = []

            def w2block(j):
                for md in range(4):
                    nc.tensor.matmul(
                        w2ps[md],
                        w2sb[:, ((e * 4 + j) * 4 + md) * 128:((e * 4 + j) * 4 + md + 1) * 128],
                        hbuf[j],
                        start=(j == 0),
                        stop=(j == 3),
                    )

            for m in range(4):
                ps = hPs.tile([128, wd], F32, name="hps")
                for kc in range(nk):
                    nc.tensor.matmul(
                        ps,
                        w1sb[e][:, (kc * 4 + m) * 128:(kc * 4 + m + 1) * 128],
                        xt[:, kc, :],
                        start=(kc == 0),
                        stop=(kc == nk - 1),
                    )
                h = hP.tile([128, wd], BF16, name="h")
                nc.scalar.activation(
                    h, ps, mybir.ActivationFunctionType.Relu,
                    bias=b1sb[:, e * 4 + m:e * 4 + m + 1], scale=1.0,
                )
                hbuf.append(h)
                if m >= 1:
                    w2block(m - 1)
            w2block(3)

            # sel = W2 out + b2, cast to bf16
            selt = selP.tile([128, 4, wd], BF16, name="selt")
            for md in range(4):
                nc.scalar.activation(
                    selt[:, md, :], w2ps[md], mybir.ActivationFunctionType.Identity,
                    bias=b2sb[:, e * 4 + md:e * 4 + md + 1], scale=1.0,
                )
            return selt

        def emit_dec(selt, goff, wd):
            # dec1 + sigmoid
            sig4 = []
            for mh in range(4):
                ps = hPs.tile([128, wd], F32, name="hps")
                for kd in range(4):
                    nc.tensor.matmul(
                        ps,
                        dw1sb[:, (kd * 4 + mh) * 128:(kd * 4 + mh + 1) * 128],
                        selt[:, kd, :],
                        start=(kd == 0),
                        stop=(kd == 3),
                    )
                sg = sigP.tile([128, wd], BF16, name="sg")
                nc.scalar.activation(
                    sg, ps, mybir.ActivationFunctionType.Sigmoid,
                    bias=db1sb[:, mh:mh + 1], scale=1.0,
                )
                sig4.append(sg)

            # dec2: out[tok, cls] per 128-token subtile (db2 added on host)
            for s in range(wd // 128):
                for n in range(NCH):
                    nw = min(512, NCLS - n * 512)
                    ps2 = d2Ps.tile([128, 512], F32, name="d2ps")
                    for kh in range(4):
                        nc.tensor.matmul(
                            ps2[:, :nw],
                            sig4[kh][:, s * 128:(s + 1) * 128],
                            dwsb[:, kh, n * 512:n * 512 + nw],
                            start=(kh == 0),
                            stop=(kh == 3),
                        )
                    ot = outP.tile([128, 512], F32, name="ot")
                    nc.vector.tensor_copy(out=ot[:, :nw], in_=ps2[:, :nw])
                    nc.scalar.dma_start(
                        out=outD[goff + s * 128:goff + (s + 1) * 128, n * 512:n * 512 + nw],
                        in_=ot[:, :nw],
                    )

        # software pipeline: decoder for tile t-1 is emitted after tile t's
        # expert stage, so the tensor engine never waits on an activation.
        pend = None
        for (e, goff, lo, wd) in tiles:
            selt = emit_w1w2(e, goff, lo, wd)
            if pend is not None:
                emit_dec(*pend)
            pend = (selt, goff, wd)
        emit_dec(*pend)

    import bass_rust

    bass_rust.generate_event_semaphores(nc)
    return nc


_NC_CACHE = {}


def _get_nc(C=None):
    if C is None:
        assert _NC_CACHE, "kernel not built yet"
        return next(iter(_NC_CACHE.values()))
    key = tuple(C)
    if key not in _NC_CACHE:
        _NC_CACHE[key] = _build(list(key))
    return _NC_CACHE[key]


def _lhsT_pack(w):
    # [K, M] -> [128, (K/128) * M], kc-major then m-chunk then m-inner
    K, M = w.shape
    nk, nm = K // 128, M // 128
    return np.ascontiguousarray(
        w.reshape(nk, 128, nm, 128).transpose(1, 0, 2, 3).reshape(128, nk * M).astype(BF)
    )


def _route(inputs):
    f32 = np.float32
    x = np.asarray(inputs["fusion_hs"], f32)
    flat = np.transpose(x, (1, 0, 2)).reshape(B, L * D)
    logits = flat.astype(np.float64) @ np.asarray(inputs["gate_W"], f32).astype(
        np.float64
    ) + np.asarray(inputs["gate_b"], f32).astype(np.float64)
    am = np.argmax(logits, axis=1)
    idx = [np.nonzero(am == e)[0] for e in range(NE)]
    percore = [[idx[e][c::NCORES] for c in range(NCORES)] for e in range(NE)]
    C = [
        int(np.ceil(max(len(percore[e][c]) for c in range(NCORES)) / PAD) * PAD)
        if len(idx[e]) else 0
        for e in range(NE)
    ]
    return x, percore, C


def _pack_x(x, idxc, e, Ce):
    l0, l1 = STREAMS[e]
    nl = l1 - l0
    K = nl * D
    nk = K // 128
    n = len(idxc)
    Xe = np.zeros((K, Ce), dtype=BF)
    if n:
        Xe[:, :n] = x[l0:l1, idxc, :].transpose(0, 2, 1).reshape(K, n).astype(BF)
    blocks = []
    for lo in range(0, Ce, TW):
        wd = min(TW, Ce - lo)
        blocks.append(
            Xe[:, lo:lo + wd].reshape(nk, 128, wd).transpose(1, 0, 2).reshape(128, nk * wd)
        )
    return np.ascontiguousarray(np.concatenate(blocks, axis=1))


def _prep_inputs(inputs):
    f32 = np.float32
    x, percore, C = _route(inputs)

    w13 = np.array(inputs["e3_W1"], f32, copy=True)
    w13[: 3 * D] *= f32(np.asarray(inputs["e3_a"]).reshape(-1)[0])
    w13[3 * D:] *= f32(np.asarray(inputs["e3_b"]).reshape(-1)[0])
    w1s = [np.asarray(inputs["e0_W1"], f32), np.asarray(inputs["e1_W1"], f32),
           np.asarray(inputs["e2_W1"], f32), w13]

    common = {f"w1_{e}": _lhsT_pack(w1s[e]) for e in range(NE)}
    common["w2all"] = np.concatenate(
        [_lhsT_pack(np.asarray(inputs[f"e{e}_W2"], f32)) for e in range(NE)], axis=1
    )
    common["dw1"] = _lhsT_pack(np.asarray(inputs["dec_W1"], f32))
    common["dw2"] = np.ascontiguousarray(
        np.asarray(inputs["dec_W2"], f32).reshape(4, 128, NCLS)
        .transpose(1, 0, 2).reshape(128, 4 * NCLS).astype(BF)
    )
    common["b1p"] = np.ascontiguousarray(
        np.concatenate([np.asarray(inputs[f"e{e}_b1"], f32) for e in range(NE)])
        .reshape(16, 128).T
    )
    common["b2p"] = np.ascontiguousarray(
        np.concatenate([np.asarray(inputs[f"e{e}_b2"], f32) for e in range(NE)])
        .reshape(16, 128).T
    )
    common["db1p"] = np.ascontiguousarray(
        np.asarray(inputs["dec_b1"], f32).reshape(4, 128).T
    )

    in_maps = []
    for c in range(NCORES):
        m = dict(common)
        for e in range(NE):
            if C[e]:
                m[f"x{e}"] = _pack_x(x, percore[e][c], e, C[e])
        in_maps.append(m)
    return in_maps, percore, C


def kernel(**inputs):
    in_maps, percore, C = _prep_inputs(inputs)
    nc = _get_nc(C)
    res = run_bass_kernel_spmd(nc, in_maps, core_ids=list(range(NCORES)))
    offs = np.concatenate([[0], np.cumsum(C)])
    out = np.empty((B, NCLS), np.float32)
    for c in range(NCORES):
        r = res.results[c]["out"]
        for e in range(NE):
            ic = percore[e][c]
            if len(ic):
                out[ic] = r[offs[e]:offs[e] + len(ic)]
    out += np.asarray(inputs["dec_b2"], np.float32).reshape(1, NCLS)
    return out


# revision 4
# speedup vs baseline: 3.2909x; 1.1145x over previous
import numpy as np
import ml_dtypes
from contextlib import ExitStack

import concourse.mybir as mybir
import concourse.bass as bass
import concourse.tile as tile
from concourse.bass_utils import run_bass_kernel_spmd

# nn_Predictor (moe_routing): L=6 streams, B=16384, D=512, NC=3992, 4 experts.
# Host computes the hard gate (argmax) and routes tokens to their expert; each
# core gets an equal, 128-padded share of every expert's tokens, pre-transposed
# ([feature, token]) and packed in bf16 so the device does no transposes and
# each token runs only its own expert's W1/W2. The decoder's second matmul runs
# in fp8 DoubleRow via the tanh identity sigmoid(x) = (1+tanh(x/2))/2: dec2
# computes sum_k tanh_k*(64*w_k)/128 and the host adds dec_b2 + 0.5*colsum(W2).
L, B, D, NCLS, NE = 6, 16384, 512, 3992, 4
NCORES = 8
PAD = 128                   # per-expert per-core column padding
TW = 512                    # column tile width
NCH = (NCLS + 511) // 512   # 8 output column chunks (last = 408)
STREAMS = [(0, 3), (3, 6), (0, 6), (0, 6)]
NK = [3 * D // 128, 3 * D // 128, 6 * D // 128, 6 * D // 128]  # 12,12,24,24
W2SCALE = 64.0              # fp8 dec_W2 pre-scale (keeps values in e4m3 normals)

F32 = mybir.dt.float32
BF16 = mybir.dt.bfloat16
F8 = mybir.dt.float8e4
BF = ml_dtypes.bfloat16
F8NP = ml_dtypes.float8_e4m3


def _build(C):
    """C: per-core padded column count per expert (multiples of PAD)."""
    nc = bass.Bass("TRN2")

    xin = {
        e: nc.dram_tensor(f"x{e}", [128, NK[e] * C[e]], BF16, kind="ExternalInput")
        for e in range(NE) if C[e]
    }
    w1d = [
        nc.dram_tensor(f"w1_{e}", [128, NK[e] * 512], BF16, kind="ExternalInput")
        for e in range(NE)
    ]
    w2d = nc.dram_tensor("w2all", [128, NE * 2048], BF16, kind="ExternalInput")
    dw1d = nc.dram_tensor("dw1", [128, 2048], BF16, kind="ExternalInput")
    dw2d = nc.dram_tensor("dw2", [128, 4 * NCLS], F8, kind="ExternalInput")
    b1d = nc.dram_tensor("b1p", [128, 16], F32, kind="ExternalInput")
    b2d = nc.dram_tensor("b2p", [128, 16], F32, kind="ExternalInput")
    db1d = nc.dram_tensor("db1p", [128, 4], F32, kind="ExternalInput")
    outD = nc.dram_tensor("out", [sum(C), NCLS], F32, kind="ExternalOutput")

    # column tiles: (expert, global col offset, in-expert offset, width)
    tiles = []
    off = 0
    for e in range(NE):
        for lo in range(0, C[e], TW):
            tiles.append((e, off + lo, lo, min(TW, C[e] - lo)))
        off += C[e]

    with tile.TileContext(nc) as tc, ExitStack() as ctx:
        singles = ctx.enter_context(tc.tile_pool(name="singles", bufs=1))
        xtP = ctx.enter_context(tc.tile_pool(name="xtP", bufs=2))
        hP = ctx.enter_context(tc.tile_pool(name="hP", bufs=6))
        selP = ctx.enter_context(tc.tile_pool(name="selP", bufs=2))
        sigP = ctx.enter_context(tc.tile_pool(name="sigP", bufs=4))
        outP = ctx.enter_context(tc.tile_pool(name="outP", bufs=6))

        hPs = ctx.enter_context(tc.tile_pool(name="hPs", bufs=2, space="PSUM"))
        mPs = ctx.enter_context(tc.tile_pool(name="mPs", bufs=2, space="PSUM"))
        d2Ps = ctx.enter_context(tc.tile_pool(name="d2Ps", bufs=4, space="PSUM"))

        w1sb = [singles.tile([128, NK[e] * 512], BF16, name=f"w1sb{e}") for e in range(NE)]
        w2sb = singles.tile([128, NE * 2048], BF16)
        dw1sb = singles.tile([128, 2048], BF16)
        dwsb = singles.tile([128, 2, 2, NCLS], F8)
        b1sb = singles.tile([128, 16], F32)
        b2sb = singles.tile([128, 16], F32)
        db1sb = singles.tile([128, 4], F32)

        # Load schedule: first expert's W1 rides the sync queue, split in half
        # and interleaved with the first x tile's halves so the PE starts
        # ~4us in. dw2 is split per class chunk across both queues so tile 0's
        # dec2 is never starved; remaining experts' W1 go last on gpsimd.
        e0_ = tiles[0][0]
        h0 = (NK[e0_] // 2) * 512
        nc.sync.dma_start(out=w1sb[e0_][:, :h0], in_=w1d[e0_][:, :h0])
        nc.gpsimd.dma_start(out=b1sb, in_=b1d[:, :])
        nc.gpsimd.dma_start(out=b2sb, in_=b2d[:, :])
        nc.gpsimd.dma_start(out=db1sb, in_=db1d[:, :])
        nc.gpsimd.dma_start(out=w2sb, in_=w2d[:, :])
        nc.gpsimd.dma_start(out=dw1sb, in_=dw1d[:, :])

        def load_dw2(n):
            nw = min(512, NCLS - n * 512)
            eng = nc.sync if n < 4 else nc.gpsimd
            eng.dma_start(
                out=dwsb[:, :, :, n * 512:n * 512 + nw],
                in_=bass.AP(tensor=dw2d, offset=n * 512,
                            ap=[[4 * NCLS, 128], [NCLS, 4], [1, nw]]),
            )

        for n in range(4, NCH):
            load_dw2(n)
        seen = {e0_}
        for (e, _, _, _) in tiles:
            if e not in seen:
                seen.add(e)
                nc.gpsimd.dma_start(out=w1sb[e], in_=w1d[e][:, :])

        def emit_w1w2(ti, e, goff, lo, wd):
            nk = NK[e]
            xt = xtP.tile([128, nk, wd], BF16, name="xt")
            if ti == 0:
                nh = nk // 2
                nc.sync.dma_start(
                    out=xt[:, :nh, :],
                    in_=bass.AP(tensor=xin[e], offset=nk * lo,
                                ap=[[NK[e] * C[e], 128], [wd, nh], [1, wd]]),
                )
                nc.sync.dma_start(out=w1sb[e][:, h0:], in_=w1d[e][:, h0:])
                nc.sync.dma_start(
                    out=xt[:, nh:, :],
                    in_=bass.AP(tensor=xin[e], offset=nk * lo + nh * wd,
                                ap=[[NK[e] * C[e], 128], [wd, nk - nh], [1, wd]]),
                )
            else:
                nc.sync.dma_start(
                    out=xt,
                    in_=bass.AP(tensor=xin[e], offset=nk * lo,
                                ap=[[NK[e] * C[e], 128], [wd, nk], [1, wd]]),
                )
            if ti == 1:
                for n in range(4):
                    load_dw2(n)

            # W1 + relu + b1 for all four hidden chunks
            hbuf = []
            for m in range(4):
                ps = hPs.tile([128, wd], F32, name="hps")
                for kc in range(nk):
                    nc.tensor.matmul(
                        ps,
                        w1sb[e][:, (kc * 4 + m) * 128:(kc * 4 + m + 1) * 128],
                        xt[:, kc, :],
                        start=(kc == 0),
                        stop=(kc == nk - 1),
                    )
                h = hP.tile([128, wd], BF16, name="h")
                nc.scalar.activation(
                    h, ps, mybir.ActivationFunctionType.Relu,
                    bias=b1sb[:, e * 4 + m:e * 4 + m + 1], scale=1.0,
                )
                hbuf.append(h)

            # W2, one output chunk at a time (2 psum banks), + b2, cast bf16
            selt = selP.tile([128, 4, wd], BF16, name="selt")
            for md in range(4):
                ps = mPs.tile([128, wd], F32, name="mps")
                for m in range(4):
                    nc.tensor.matmul(
                        ps,
                        w2sb[:, ((e * 4 + m) * 4 + md) * 128:((e * 4 + m) * 4 + md + 1) * 128],
                        hbuf[m],
                        start=(m == 0),
                        stop=(m == 3),
                    )
                nc.scalar.activation(
                    selt[:, md, :], ps, mybir.ActivationFunctionType.Identity,
                    bias=b2sb[:, e * 4 + md:e * 4 + md + 1], scale=1.0,
                )
            return selt

        def emit_dec(selt, goff, wd):
            # dec1; tanh((x+db1)/2) in fp8 pairs for the DoubleRow dec2
            sigp = [sigP.tile([128, 2, wd], F8, name="sgp") for _ in range(2)]
            for mh in range(4):
                ps = mPs.tile([128, wd], F32, name="mps")
                for kd in range(4):
                    nc.tensor.matmul(
                        ps,
                        dw1sb[:, (kd * 4 + mh) * 128:(kd * 4 + mh + 1) * 128],
                        selt[:, kd, :],
                        start=(kd == 0),
                        stop=(kd == 3),
                    )
                nc.scalar.activation(
                    sigp[mh // 2][:, mh % 2, :], ps,
                    mybir.ActivationFunctionType.Tanh,
                    bias=db1sb[:, mh:mh + 1], scale=0.5,
                )

            # dec2 fp8 DoubleRow: out = sum_j sigp_j.T @ dwp_j, scaled 1/2/64
            for s in range(wd // 128):
                for n in range(NCH):
                    nw = min(512, NCLS - n * 512)
                    ps2 = d2Ps.tile([128, 512], F32, name="d2ps")
                    for j in range(2):
                        nc.tensor.matmul(
                            ps2[:, :nw],
                            sigp[j][:, :, s * 128:(s + 1) * 128],
                            dwsb[:, j, :, n * 512:n * 512 + nw],
                            start=(j == 0),
                            stop=(j == 1),
                            perf_mode=mybir.MatmulPerfMode.DoubleRow,
                        )
                    ot = outP.tile([128, 512], F32, name="ot")
                    if n % 2 == 0:
                        nc.vector.tensor_scalar_mul(ot[:, :nw], ps2[:, :nw], 1.0 / (2 * W2SCALE))
                    else:
                        nc.scalar.activation(
                            ot[:, :nw], ps2[:, :nw],
                            mybir.ActivationFunctionType.Copy, scale=1.0 / (2 * W2SCALE),
                        )
                    nc.scalar.dma_start(
                        out=outD[goff + s * 128:goff + (s + 1) * 128, n * 512:n * 512 + nw],
                        in_=ot[:, :nw],
                    )

        # software pipeline: decoder for tile t-1 is emitted after tile t's
        # expert stage, so the tensor engine never waits on an activation.
        pend = None
        for ti, (e, goff, lo, wd) in enumerate(tiles):
            selt = emit_w1w2(ti, e, goff, lo, wd)
            if pend is not None:
                emit_dec(*pend)
            pend = (selt, goff, wd)
        emit_dec(*pend)

    import bass_rust

    bass_rust.generate_event_semaphores(nc)
    return nc


_NC_CACHE = {}


def _get_nc(C=None):
    if C is None:
        assert _NC_CACHE, "kernel not built yet"
        return next(iter(_NC_CACHE.values()))
    key = tuple(C)
    if key not in _NC_CACHE:
        _NC_CACHE[key] = _build(list(key))
    return _NC_CACHE[key]


def _lhsT_pack(w):
    # [K, M] -> [128, (K/128) * M], kc-major then m-chunk then m-inner
    K, M = w.shape
    nk, nm = K // 128, M // 128
    return np.ascontiguousarray(
        w.reshape(nk, 128, nm, 128).transpose(1, 0, 2, 3).reshape(128, nk * M).astype(BF)
    )


def _route(inputs):
    f32 = np.float32
    x = np.asarray(inputs["fusion_hs"], f32)
    flat = np.transpose(x, (1, 0, 2)).reshape(B, L * D)
    logits = flat.astype(np.float64) @ np.asarray(inputs["gate_W"], f32).astype(
        np.float64
    ) + np.asarray(inputs["gate_b"], f32).astype(np.float64)
    am = np.argmax(logits, axis=1)
    idx = [np.nonzero(am == e)[0] for e in range(NE)]
    percore = [[idx[e][c::NCORES] for c in range(NCORES)] for e in range(NE)]
    C = [
        int(np.ceil(max(len(percore[e][c]) for c in range(NCORES)) / PAD) * PAD)
        if len(idx[e]) else 0
        for e in range(NE)
    ]
    return x, percore, C


def _pack_x(x, idxc, e, Ce):
    l0, l1 = STREAMS[e]
    nl = l1 - l0
    K = nl * D
    nk = K // 128
    n = len(idxc)
    Xe = np.zeros((K, Ce), dtype=BF)
    if n:
        Xe[:, :n] = x[l0:l1, idxc, :].transpose(0, 2, 1).reshape(K, n).astype(BF)
    blocks = []
    for lo in range(0, Ce, TW):
        wd = min(TW, Ce - lo)
        blocks.append(
            Xe[:, lo:lo + wd].reshape(nk, 128, wd).transpose(1, 0, 2).reshape(128, nk * wd)
        )
    return np.ascontiguousarray(np.concatenate(blocks, axis=1))


def _prep_inputs(inputs):
    f32 = np.float32
    x, percore, C = _route(inputs)

    w13 = np.array(inputs["e3_W1"], f32, copy=True)
    w13[: 3 * D] *= f32(np.asarray(inputs["e3_a"]).reshape(-1)[0])
    w13[3 * D:] *= f32(np.asarray(inputs["e3_b"]).reshape(-1)[0])
    w1s = [np.asarray(inputs["e0_W1"], f32), np.asarray(inputs["e1_W1"], f32),
           np.asarray(inputs["e2_W1"], f32), w13]

    common = {f"w1_{e}": _lhsT_pack(w1s[e]) for e in range(NE)}
    common["w2all"] = np.concatenate(
        [_lhsT_pack(np.asarray(inputs[f"e{e}_W2"], f32)) for e in range(NE)], axis=1
    )
    common["dw1"] = _lhsT_pack(np.asarray(inputs["dec_W1"], f32))
    dw2 = np.asarray(inputs["dec_W2"], f32)
    common["dw2"] = np.ascontiguousarray(
        (dw2 * W2SCALE).reshape(2, 2, 128, NCLS).transpose(2, 0, 1, 3)
        .reshape(128, 4 * NCLS).astype(F8NP)
    )
    common["b1p"] = np.ascontiguousarray(
        np.concatenate([np.asarray(inputs[f"e{e}_b1"], f32) for e in range(NE)])
        .reshape(16, 128).T
    )
    common["b2p"] = np.ascontiguousarray(
        np.concatenate([np.asarray(inputs[f"e{e}_b2"], f32) for e in range(NE)])
        .reshape(16, 128).T
    )
    common["db1p"] = np.ascontiguousarray(
        (np.asarray(inputs["dec_b1"], f32) * 0.5).reshape(4, 128).T
    )

    in_maps = []
    for c in range(NCORES):
        m = dict(common)
        for e in range(NE):
            if C[e]:
                m[f"x{e}"] = _pack_x(x, percore[e][c], e, C[e])
        in_maps.append(m)
    return in_maps, percore, C


def kernel(**inputs):
    in_maps, percore, C = _prep_inputs(inputs)
    nc = _get_nc(C)
    res = run_bass_kernel_spmd(nc, in_maps, core_ids=list(range(NCORES)))
    offs = np.concatenate([[0], np.cumsum(C)])
    out = np.empty((B, NCLS), np.float32)
    for c in range(NCORES):
        r = res.results[c]["out"]
        for e in range(NE):
            ic = percore[e][c]
            if len(ic):
                out[ic] = r[offs[e]:offs[e] + len(ic)]
    # dec2 ran on tanh/2 - 0.5 centered activations; fold the 0.5*colsum(W2)
    # constant and dec_b2 back in here.
    dw2 = np.asarray(inputs["dec_W2"], np.float64)
    out += (np.asarray(inputs["dec_b2"], np.float64).reshape(1, NCLS)
            + 0.5 * dw2.sum(axis=0).reshape(1, NCLS)).astype(np.float32)
    return out


# revision 13
# speedup vs baseline: 3.6377x; 1.1054x over previous
import numpy as np
import ml_dtypes
from contextlib import ExitStack

import concourse.mybir as mybir
import concourse.bass as bass
import concourse.tile as tile
from concourse.bass_utils import run_bass_kernel_spmd

# nn_Predictor (moe_routing): L=6 streams, B=16384, D=512, NC=3992, 4 experts.
# Host computes the hard gate (argmax) and routes tokens to their expert; each
# core gets an equal, 128-padded share of every expert's tokens, pre-transposed
# ([feature, token]) and packed in bf16 so the device does no transposes and
# each token runs only its own expert's W1/W2. The decoder's second matmul runs
# in fp8 DoubleRow via the tanh identity sigmoid(x) = (1+tanh(x/2))/2: dec2
# computes sum_k tanh_k*(64*w_k)/128 and the host adds dec_b2 + 0.5*colsum(W2).
L, B, D, NCLS, NE = 6, 16384, 512, 3992, 4
NCORES = 8
PAD = 128                   # per-expert per-core column padding
TW = 512                    # column tile width
NCH = (NCLS + 511) // 512   # 8 output column chunks (last = 408)
STREAMS = [(0, 3), (3, 6), (0, 6), (0, 6)]
NK = [3 * D // 128, 3 * D // 128, 6 * D // 128, 6 * D // 128]  # 12,12,24,24
W2SCALE = 64.0              # fp8 dec_W2 pre-scale (keeps values in e4m3 normals)

F32 = mybir.dt.float32
BF16 = mybir.dt.bfloat16
F8 = mybir.dt.float8e4
BF = ml_dtypes.bfloat16
F8NP = ml_dtypes.float8_e4m3


def _build(C):
    """C: per-core padded column count per expert (multiples of PAD)."""
    nc = bass.Bass("TRN2")

    xin = {
        e: nc.dram_tensor(f"x{e}", [128, NK[e] * C[e]], BF16, kind="ExternalInput")
        for e in range(NE) if C[e]
    }
    w1d = [
        nc.dram_tensor(f"w1_{e}", [128, NK[e] * 512], BF16, kind="ExternalInput")
        for e in range(NE)
    ]
    w2d = nc.dram_tensor("w2all", [128, NE * 2048], BF16, kind="ExternalInput")
    dw1d = nc.dram_tensor("dw1", [128, 2048], BF16, kind="ExternalInput")
    dw2d = nc.dram_tensor("dw2", [128, 4 * NCLS], F8, kind="ExternalInput")
    biasd = nc.dram_tensor("biasp", [128, 36], F32, kind="ExternalInput")
    outD = nc.dram_tensor("out", [sum(C), NCLS], F32, kind="ExternalOutput")

    # column tiles: (expert, global col offset, in-expert offset, width)
    tiles = []
    off = 0
    for e in range(NE):
        for lo in range(0, C[e], TW):
            tiles.append((e, off + lo, lo, min(TW, C[e] - lo)))
        off += C[e]

    with tile.TileContext(nc) as tc, ExitStack() as ctx:
        singles = ctx.enter_context(tc.tile_pool(name="singles", bufs=1))
        xtP = ctx.enter_context(tc.tile_pool(name="xtP", bufs=2))
        hP = ctx.enter_context(tc.tile_pool(name="hP", bufs=4))
        selP = ctx.enter_context(tc.tile_pool(name="selP", bufs=2))
        sigP = ctx.enter_context(tc.tile_pool(name="sigP", bufs=4))
        obP = ctx.enter_context(tc.tile_pool(name="obP", bufs=3))

        hPs = ctx.enter_context(tc.tile_pool(name="hPs", bufs=2, space="PSUM"))
        mPs = ctx.enter_context(tc.tile_pool(name="mPs", bufs=2, space="PSUM"))
        d2Ps = ctx.enter_context(tc.tile_pool(name="d2Ps", bufs=4, space="PSUM"))

        w1sb = [singles.tile([128, NK[e] * 512], BF16, name=f"w1sb{e}") for e in range(NE)]
        w2sb = singles.tile([128, NE * 2048], BF16)
        dw1sb = singles.tile([128, 2048], BF16)
        dwsb = singles.tile([128, 2, 2, NCLS], F8)
        biassb = singles.tile([128, 36], F32)

        # Load schedule. sync queue: biases + first expert's W1 (halved and
        # interleaved with the first x tile) + the per-tile x loads. scalar
        # queue (idle until the first stores ~35us in): w2/dw1/dw2 + the
        # second expert's W1, ordered by first use. gpsimd SWDGE is slow
        # (~250GB/s with ~us descriptor-gen stalls) so it only gets the last
        # two experts' W1, whose deadlines are ~100us out.
        e0_ = tiles[0][0]
        h0 = (NK[e0_] // 2) * 512
        nc.sync.dma_start(out=biassb, in_=biasd[:, :])
        nc.sync.dma_start(out=w1sb[e0_][:, :h0], in_=w1d[e0_][:, :h0])
        nc.scalar.dma_start(out=w2sb, in_=w2d[:, :])
        nc.scalar.dma_start(out=dw1sb, in_=dw1d[:, :])
        for n in range(NCH):
            nw = min(512, NCLS - n * 512)
            nc.scalar.dma_start(
                out=dwsb[:, :, :, n * 512:n * 512 + nw],
                in_=bass.AP(tensor=dw2d, offset=n * 512,
                            ap=[[4 * NCLS, 128], [NCLS, 4], [1, nw]]),
            )
        rest = []
        seen = {e0_}
        for (e, _, _, _) in tiles:
            if e not in seen:
                seen.add(e)
                rest.append(e)
        for i, e in enumerate(rest):
            eng = nc.scalar if i == 0 else nc.gpsimd
            eng.dma_start(out=w1sb[e], in_=w1d[e][:, :])

        def emit_w1w2(ti, e, goff, lo, wd):
            nk = NK[e]
            xt = xtP.tile([128, nk, wd], BF16, name="xt")
            if ti == 0:
                nh = nk // 2
                nc.sync.dma_start(
                    out=xt[:, :nh, :],
                    in_=bass.AP(tensor=xin[e], offset=nk * lo,
                                ap=[[NK[e] * C[e], 128], [wd, nh], [1, wd]]),
                )
                nc.sync.dma_start(out=w1sb[e][:, h0:], in_=w1d[e][:, h0:])
                nc.sync.dma_start(
                    out=xt[:, nh:, :],
                    in_=bass.AP(tensor=xin[e], offset=nk * lo + nh * wd,
                                ap=[[NK[e] * C[e], 128], [wd, nk - nh], [1, wd]]),
                )
            else:
                nc.sync.dma_start(
                    out=xt,
                    in_=bass.AP(tensor=xin[e], offset=nk * lo,
                                ap=[[NK[e] * C[e], 128], [wd, nk], [1, wd]]),
                )

            # W1 + relu + b1 for all four hidden chunks
            hbuf = []
            for m in range(4):
                ps = hPs.tile([128, wd], F32, name="hps")
                for kc in range(nk):
                    nc.tensor.matmul(
                        ps,
                        w1sb[e][:, (kc * 4 + m) * 128:(kc * 4 + m + 1) * 128],
                        xt[:, kc, :],
                        start=(kc == 0),
                        stop=(kc == nk - 1),
                    )
                h = hP.tile([128, wd], BF16, name="h")
                nc.scalar.activation(
                    h, ps, mybir.ActivationFunctionType.Relu,
                    bias=biassb[:, e * 4 + m:e * 4 + m + 1], scale=1.0,
                )
                hbuf.append(h)

            # W2, one output chunk at a time (2 psum banks), + b2, cast bf16
            selt = selP.tile([128, 4, wd], BF16, name="selt")
            for md in range(4):
                ps = mPs.tile([128, wd], F32, name="mps")
                for m in range(4):
                    nc.tensor.matmul(
                        ps,
                        w2sb[:, ((e * 4 + m) * 4 + md) * 128:((e * 4 + m) * 4 + md + 1) * 128],
                        hbuf[m],
                        start=(m == 0),
                        stop=(m == 3),
                    )
                nc.scalar.activation(
                    selt[:, md, :], ps, mybir.ActivationFunctionType.Identity,
                    bias=biassb[:, 16 + e * 4 + md:16 + e * 4 + md + 1], scale=1.0,
                )
            return selt

        def emit_dec(selt, goff, wd):
            # dec1; tanh((x+db1)/2) in fp8 pairs for the DoubleRow dec2
            sigp = [sigP.tile([128, 2, wd], F8, name="sgp") for _ in range(2)]
            for mh in range(4):
                ps = mPs.tile([128, wd], F32, name="mps")
                for kd in range(4):
                    nc.tensor.matmul(
                        ps,
                        dw1sb[:, (kd * 4 + mh) * 128:(kd * 4 + mh + 1) * 128],
                        selt[:, kd, :],
                        start=(kd == 0),
                        stop=(kd == 3),
                    )
                nc.scalar.activation(
                    sigp[mh // 2][:, mh % 2, :], ps,
                    mybir.ActivationFunctionType.Tanh,
                    bias=biassb[:, 32 + mh:32 + mh + 1], scale=0.5,
                )

            # dec2 fp8 DoubleRow: out = sum_j sigp_j.T @ dwp_j, scaled 1/2/64.
            # Class chunks accumulate into a [128, 2048] staging half-row so
            # each token subtile needs only two store triggers, not eight.
            for s in range(wd // 128):
                for half in range(2):
                    nws = 2048 if half == 0 else NCLS - 2048
                    ob = obP.tile([128, 2048], F32, name="ob")
                    for ni in range(4):
                        n = half * 4 + ni
                        nw = min(512, NCLS - n * 512)
                        ps2 = d2Ps.tile([128, 512], F32, name="d2ps")
                        for j in range(2):
                            nc.tensor.matmul(
                                ps2[:, :nw],
                                sigp[j][:, :, s * 128:(s + 1) * 128],
                                dwsb[:, j, :, n * 512:n * 512 + nw],
                                start=(j == 0),
                                stop=(j == 1),
                                perf_mode=mybir.MatmulPerfMode.DoubleRow,
                            )
                        dst = ob[:, ni * 512:ni * 512 + nw]
                        if n % 2 == 0:
                            nc.vector.tensor_scalar_mul(dst, ps2[:, :nw], 1.0 / (2 * W2SCALE))
                        else:
                            nc.scalar.activation(
                                dst, ps2[:, :nw],
                                mybir.ActivationFunctionType.Copy, scale=1.0 / (2 * W2SCALE),
                            )
                    nc.scalar.dma_start(
                        out=outD[goff + s * 128:goff + (s + 1) * 128,
                                 half * 2048:half * 2048 + nws],
                        in_=ob[:, :nws],
                    )

        # software pipeline: decoder for tile t-1 is emitted after tile t's
        # expert stage, so the tensor engine never waits on an activation.
        pend = None
        for ti, (e, goff, lo, wd) in enumerate(tiles):
            selt = emit_w1w2(ti, e, goff, lo, wd)
            if pend is not None:
                emit_dec(*pend)
            pend = (selt, goff, wd)
        emit_dec(*pend)

    import bass_rust

    bass_rust.generate_event_semaphores(nc)
    return nc


_NC_CACHE = {}


def _get_nc(C=None):
    if C is None:
        assert _NC_CACHE, "kernel not built yet"
        return next(iter(_NC_CACHE.values()))
    key = tuple(C)
    if key not in _NC_CACHE:
        _NC_CACHE[key] = _build(list(key))
    return _NC_CACHE[key]


def _lhsT_pack(w):
    # [K, M] -> [128, (K/128) * M], kc-major then m-chunk then m-inner
    K, M = w.shape
    nk, nm = K // 128, M // 128
    return np.ascontiguousarray(
        w.reshape(nk, 128, nm, 128).transpose(1, 0, 2, 3).reshape(128, nk * M).astype(BF)
    )


def _route(inputs):
    f32 = np.float32
    x = np.asarray(inputs["fusion_hs"], f32)
    flat = np.transpose(x, (1, 0, 2)).reshape(B, L * D)
    logits = flat.astype(np.float64) @ np.asarray(inputs["gate_W"], f32).astype(
        np.float64
    ) + np.asarray(inputs["gate_b"], f32).astype(np.float64)
    am = np.argmax(logits, axis=1)
    idx = [np.nonzero(am == e)[0] for e in range(NE)]
    percore = [[idx[e][c::NCORES] for c in range(NCORES)] for e in range(NE)]
    C = [
        int(np.ceil(max(len(percore[e][c]) for c in range(NCORES)) / PAD) * PAD)
        if len(idx[e]) else 0
        for e in range(NE)
    ]
    return x, percore, C


def _pack_x(x, idxc, e, Ce):
    l0, l1 = STREAMS[e]
    nl = l1 - l0
    K = nl * D
    nk = K // 128
    n = len(idxc)
    Xe = np.zeros((K, Ce), dtype=BF)
    if n:
        Xe[:, :n] = x[l0:l1, idxc, :].transpose(0, 2, 1).reshape(K, n).astype(BF)
    blocks = []
    for lo in range(0, Ce, TW):
        wd = min(TW, Ce - lo)
        blocks.append(
            Xe[:, lo:lo + wd].reshape(nk, 128, wd).transpose(1, 0, 2).reshape(128, nk * wd)
        )
    return np.ascontiguousarray(np.concatenate(blocks, axis=1))


def _prep_inputs(inputs):
    f32 = np.float32
    x, percore, C = _route(inputs)

    w13 = np.array(inputs["e3_W1"], f32, copy=True)
    w13[: 3 * D] *= f32(np.asarray(inputs["e3_a"]).reshape(-1)[0])
    w13[3 * D:] *= f32(np.asarray(inputs["e3_b"]).reshape(-1)[0])
    w1s = [np.asarray(inputs["e0_W1"], f32), np.asarray(inputs["e1_W1"], f32),
           np.asarray(inputs["e2_W1"], f32), w13]

    common = {f"w1_{e}": _lhsT_pack(w1s[e]) for e in range(NE)}
    common["w2all"] = np.concatenate(
        [_lhsT_pack(np.asarray(inputs[f"e{e}_W2"], f32)) for e in range(NE)], axis=1
    )
    common["dw1"] = _lhsT_pack(np.asarray(inputs["dec_W1"], f32))
    dw2 = np.asarray(inputs["dec_W2"], f32)
    common["dw2"] = np.ascontiguousarray(
        (dw2 * W2SCALE).reshape(2, 2, 128, NCLS).transpose(2, 0, 1, 3)
        .reshape(128, 4 * NCLS).astype(F8NP)
    )
    b1p = np.concatenate(
        [np.asarray(inputs[f"e{e}_b1"], f32) for e in range(NE)]
    ).reshape(16, 128).T
    b2p = np.concatenate(
        [np.asarray(inputs[f"e{e}_b2"], f32) for e in range(NE)]
    ).reshape(16, 128).T
    db1p = (np.asarray(inputs["dec_b1"], f32) * 0.5).reshape(4, 128).T
    common["biasp"] = np.ascontiguousarray(
        np.concatenate([b1p, b2p, db1p], axis=1)
    )

    in_maps = []
    for c in range(NCORES):
        m = dict(common)
        for e in range(NE):
            if C[e]:
                m[f"x{e}"] = _pack_x(x, percore[e][c], e, C[e])
        in_maps.append(m)
    return in_maps, percore, C


def kernel(**inputs):
    in_maps, percore, C = _prep_inputs(inputs)
    nc = _get_nc(C)
    res = run_bass_kernel_spmd(nc, in_maps, core_ids=list(range(NCORES)))
    offs = np.concatenate([[0], np.cumsum(C)])
    out = np.empty((B, NCLS), np.float32)
    for c in range(NCORES):
        r = res.results[c]["out"]
        for e in range(NE):
            ic = percore[e][c]
            if len(ic):
                out[ic] = r[offs[e]:offs[e] + len(ic)]
    # dec2 ran on tanh/2 - 0.5 centered activations; fold the 0.5*colsum(W2)
    # constant and dec_b2 back in here.
    dw2 = np.asarray(inputs["dec_W2"], np.float64)
    out += (np.asarray(inputs["dec_b2"], np.float64).reshape(1, NCLS)
            + 0.5 * dw2.sum(axis=0).reshape(1, NCLS)).astype(np.float32)
    return out


# revision 16
# speedup vs baseline: 4.6744x; 1.2850x over previous
import numpy as np
import ml_dtypes
from contextlib import ExitStack

import concourse.mybir as mybir
import concourse.bass as bass
import concourse.tile as tile
from concourse.bass_utils import run_bass_kernel_spmd

# nn_Predictor (moe_routing): L=6 streams, B=16384, D=512, NC=3992, 4 experts.
# Host computes the hard gate (argmax) and routes tokens to their expert; each
# core gets an equal, 128-padded share of every expert's tokens, pre-transposed
# ([feature, token]) and packed in fp8 K-pairs so every matmul runs in fp8
# DoubleRow (2x PE throughput) with no on-device transposes. Weights are
# pre-scaled into e4m3's normal range (x32/x64) and the scales divided back
# out in the psum->sbuf activations, whose f32 biases keep b1/b2 exact. The
# decoder sigmoid uses sigmoid(x) = (1+tanh(x/2))/2 so dec2 multiplies
# centered tanh values; the host adds dec_b2 + 0.5*colsum(dec_W2) at the end.
L, B, D, NCLS, NE = 6, 16384, 512, 3992, 4
NCORES = 8
PAD = 128                   # per-expert per-core column padding
TW = 512                    # column tile width
NCH = (NCLS + 511) // 512   # 8 output column chunks (last = 408)
STREAMS = [(0, 3), (3, 6), (0, 6), (0, 6)]
NK = [3 * D // 128, 3 * D // 128, 6 * D // 128, 6 * D // 128]  # 12,12,24,24
W1S = 32.0                  # fp8 pre-scales
W2S = 64.0
DW1S = 64.0
DW2S = 64.0

F32 = mybir.dt.float32
F8 = mybir.dt.float8e4
F8NP = ml_dtypes.float8_e4m3
DR = mybir.MatmulPerfMode.DoubleRow


def _build(C):
    """C: per-core padded column count per expert (multiples of PAD)."""
    nc = bass.Bass("TRN2")

    xin = {
        e: nc.dram_tensor(f"x{e}", [128, NK[e] * C[e]], F8, kind="ExternalInput")
        for e in range(NE) if C[e]
    }
    w1d = [
        nc.dram_tensor(f"w1_{e}", [128, NK[e] * 512], F8, kind="ExternalInput")
        for e in range(NE)
    ]
    w2d = nc.dram_tensor("w2all", [128, NE * 2048], F8, kind="ExternalInput")
    dw1d = nc.dram_tensor("dw1", [128, 2048], F8, kind="ExternalInput")
    dw2d = nc.dram_tensor("dw2", [128, 4 * NCLS], F8, kind="ExternalInput")
    biasd = nc.dram_tensor("biasp", [128, 36], F32, kind="ExternalInput")
    outD = nc.dram_tensor("out", [sum(C), NCLS], F32, kind="ExternalOutput")

    # column tiles: (expert, global col offset, in-expert offset, width)
    tiles = []
    off = 0
    for e in range(NE):
        for lo in range(0, C[e], TW):
            tiles.append((e, off + lo, lo, min(TW, C[e] - lo)))
        off += C[e]

    with tile.TileContext(nc) as tc, ExitStack() as ctx:
        singles = ctx.enter_context(tc.tile_pool(name="singles", bufs=1))
        xtP = ctx.enter_context(tc.tile_pool(name="xtP", bufs=3))
        hP = ctx.enter_context(tc.tile_pool(name="hP", bufs=4))
        selP = ctx.enter_context(tc.tile_pool(name="selP", bufs=4))
        sigP = ctx.enter_context(tc.tile_pool(name="sigP", bufs=4))
        obP = ctx.enter_context(tc.tile_pool(name="obP", bufs=4))

        hPs = ctx.enter_context(tc.tile_pool(name="hPs", bufs=2, space="PSUM"))
        mPs = ctx.enter_context(tc.tile_pool(name="mPs", bufs=2, space="PSUM"))
        d2Ps = ctx.enter_context(tc.tile_pool(name="d2Ps", bufs=4, space="PSUM"))

        # fp8 pair layouts: lhsT slices are [128, 2, 128], rhs [128, 2, wd]
        w1sb = [
            singles.tile([128, NK[e] // 2, 4, 2, 128], F8, name=f"w1sb{e}")
            for e in range(NE)
        ]
        w2sb = singles.tile([128, NE, 2, 4, 2, 128], F8)
        dw1sb = singles.tile([128, 2, 4, 2, 128], F8)
        dwsb = singles.tile([128, 2, 2, NCLS], F8)
        biassb = singles.tile([128, 36], F32)

        # Load schedule: every startup-critical load rides the sync queue in
        # strict deadline order (in-queue order is priority; concurrent queues
        # round-robin per descriptor and starve small early loads). Weight
        # loads are interleaved between tile emissions; late x tiles go to
        # gpsimd SWDGE, whose slow drain meets their distant deadlines.
        e0_ = tiles[0][0]
        hp0 = NK[e0_] // 4          # half the pairs of the first expert's W1
        nc.sync.dma_start(out=biassb, in_=biasd[:, :])
        nc.sync.dma_start(out=w1sb[e0_][:, :hp0], in_=w1d[e0_][:, :hp0 * 1024])

        rest = []
        seen = {e0_}
        for (e, _, _, _) in tiles:
            if e not in seen:
                seen.add(e)
                rest.append(e)

        def post_tile_loads(ti):
            if ti == 0:
                nc.sync.dma_start(out=w2sb, in_=w2d[:, :])
                nc.sync.dma_start(out=dw1sb, in_=dw1d[:, :])
            elif ti == 1:
                nc.sync.dma_start(
                    out=dwsb,
                    in_=bass.AP(tensor=dw2d, offset=0,
                                ap=[[4 * NCLS, 128], [NCLS, 4], [1, NCLS]]),
                )
                if len(rest) > 0:
                    e = rest[0]
                    nc.sync.dma_start(out=w1sb[e], in_=w1d[e][:, :])
            elif ti == 2:
                for e in rest[1:]:
                    nc.sync.dma_start(out=w1sb[e], in_=w1d[e][:, :])

        def emit_w1w2(ti, e, goff, lo, wd):
            nk = NK[e]
            npair = nk // 2
            xt = xtP.tile([128, npair, 2, wd], F8, name="xt")
            xeng = nc.sync if ti <= 2 else nc.gpsimd
            if ti == 0:
                nh = npair // 2
                nc.sync.dma_start(
                    out=xt[:, :nh, :, :],
                    in_=bass.AP(tensor=xin[e], offset=nk * lo,
                                ap=[[NK[e] * C[e], 128], [2 * wd, nh], [wd, 2], [1, wd]]),
                )
                nc.sync.dma_start(
                    out=w1sb[e][:, hp0:], in_=w1d[e][:, hp0 * 1024:]
                )
                nc.sync.dma_start(
                    out=xt[:, nh:, :, :],
                    in_=bass.AP(tensor=xin[e], offset=nk * lo + nh * 2 * wd,
                                ap=[[NK[e] * C[e], 128], [2 * wd, npair - nh], [wd, 2], [1, wd]]),
                )
            else:
                xeng.dma_start(
                    out=xt,
                    in_=bass.AP(tensor=xin[e], offset=nk * lo,
                                ap=[[NK[e] * C[e], 128], [2 * wd, npair], [wd, 2], [1, wd]]),
                )
            post_tile_loads(ti)

            # W1 (DoubleRow) + relu; h written as x32-scaled fp8 pairs
            hp = [hP.tile([128, 2, wd], F8, name="hp") for _ in range(2)]
            for m in range(4):
                ps = hPs.tile([128, wd], F32, name="hps")
                for p in range(npair):
                    nc.tensor.matmul(
                        ps,
                        w1sb[e][:, p, m],
                        xt[:, p],
                        start=(p == 0),
                        stop=(p == npair - 1),
                        perf_mode=DR,
                    )
                nc.scalar.activation(
                    hp[m // 2][:, m % 2, :], ps, mybir.ActivationFunctionType.Relu,
                    bias=biassb[:, e * 4 + m:e * 4 + m + 1], scale=1.0,
                )

            # W2 (DoubleRow) + b2, scale 1/(32*64) divided out, sel as fp8 pairs
            selp = [selP.tile([128, 2, wd], F8, name="selp") for _ in range(2)]
            for md in range(4):
                ps = mPs.tile([128, wd], F32, name="mps")
                for j in range(2):
                    nc.tensor.matmul(
                        ps, w2sb[:, e, j, md], hp[j],
                        start=(j == 0), stop=(j == 1), perf_mode=DR,
                    )
                nc.scalar.activation(
                    selp[md // 2][:, md % 2, :], ps,
                    mybir.ActivationFunctionType.Identity,
                    bias=biassb[:, 16 + e * 4 + md:16 + e * 4 + md + 1],
                    scale=1.0 / (W1S * W2S),
                )
            return selp

        def emit_dec(selp, goff, wd):
            # dec1 (DoubleRow, x64); tanh((z+db1)/2) in fp8 pairs for dec2
            sigp = [sigP.tile([128, 2, wd], F8, name="sgp") for _ in range(2)]
            for mh in range(4):
                ps = mPs.tile([128, wd], F32, name="mps")
                for j in range(2):
                    nc.tensor.matmul(
                        ps, dw1sb[:, j, mh], selp[j],
                        start=(j == 0), stop=(j == 1), perf_mode=DR,
                    )
                nc.scalar.activation(
                    sigp[mh // 2][:, mh % 2, :], ps,
                    mybir.ActivationFunctionType.Tanh,
                    bias=biassb[:, 32 + mh:32 + mh + 1], scale=0.5 / DW1S,
                )

            # dec2 fp8 DoubleRow; class chunks land in a [128, 2048] staging
            # half-row so each token subtile needs two store triggers, not 8
            for s in range(wd // 128):
                for half in range(2):
                    nws = 2048 if half == 0 else NCLS - 2048
                    ob = obP.tile([128, 2048], F32, name="ob")
                    for ni in range(4):
                        n = half * 4 + ni
                        nw = min(512, NCLS - n * 512)
                        ps2 = d2Ps.tile([128, 512], F32, name="d2ps")
                        for j in range(2):
                            nc.tensor.matmul(
                                ps2[:, :nw],
                                sigp[j][:, :, s * 128:(s + 1) * 128],
                                dwsb[:, j, :, n * 512:n * 512 + nw],
                                start=(j == 0),
                                stop=(j == 1),
                                perf_mode=DR,
                            )
                        dst = ob[:, ni * 512:ni * 512 + nw]
                        if n % 2 == 0:
                            nc.vector.tensor_scalar_mul(dst, ps2[:, :nw], 1.0 / (2 * DW2S))
                        else:
                            nc.scalar.activation(
                                dst, ps2[:, :nw],
                                mybir.ActivationFunctionType.Copy, scale=1.0 / (2 * DW2S),
                            )
                    nc.scalar.dma_start(
                        out=outD[goff + s * 128:goff + (s + 1) * 128,
                                 half * 2048:half * 2048 + nws],
                        in_=ob[:, :nws],
                    )

        # software pipeline: decoder for tile t-1 is emitted after tile t's
        # expert stage, so the tensor engine never waits on an activation.
        pend = None
        for ti, (e, goff, lo, wd) in enumerate(tiles):
            selp = emit_w1w2(ti, e, goff, lo, wd)
            if pend is not None:
                emit_dec(*pend)
            pend = (selp, goff, wd)
        emit_dec(*pend)

    import bass_rust

    bass_rust.generate_event_semaphores(nc)
    return nc


_NC_CACHE = {}


def _get_nc(C=None):
    if C is None:
        assert _NC_CACHE, "kernel not built yet"
        return next(iter(_NC_CACHE.values()))
    key = tuple(C)
    if key not in _NC_CACHE:
        _NC_CACHE[key] = _build(list(key))
    return _NC_CACHE[key]


def _pair_pack(w, scale):
    # [K, M] -> [128, K/256, M/128, 2, 128] fp8 pairs, flattened to 2D
    K, M = w.shape
    npair, nm = K // 256, M // 128
    return np.ascontiguousarray(
        (w * scale).reshape(npair, 2, 128, nm, 128).transpose(2, 0, 3, 1, 4)
        .reshape(128, K * M // 128).astype(F8NP)
    )


def _route(inputs):
    f32 = np.float32
    x = np.asarray(inputs["fusion_hs"], f32)
    flat = np.transpose(x, (1, 0, 2)).reshape(B, L * D)
    logits = flat.astype(np.float64) @ np.asarray(inputs["gate_W"], f32).astype(
        np.float64
    ) + np.asarray(inputs["gate_b"], f32).astype(np.float64)
    am = np.argmax(logits, axis=1)
    idx = [np.nonzero(am == e)[0] for e in range(NE)]
    percore = [[idx[e][c::NCORES] for c in range(NCORES)] for e in range(NE)]
    C = [
        int(np.ceil(max(len(percore[e][c]) for c in range(NCORES)) / PAD) * PAD)
        if len(idx[e]) else 0
        for e in range(NE)
    ]
    return x, percore, C


def _pack_x(x, idxc, e, Ce):
    l0, l1 = STREAMS[e]
    nl = l1 - l0
    K = nl * D
    nk = K // 128
    n = len(idxc)
    Xe = np.zeros((K, Ce), dtype=F8NP)
    if n:
        Xe[:, :n] = x[l0:l1, idxc, :].transpose(0, 2, 1).reshape(K, n).astype(F8NP)
    blocks = []
    for lo in range(0, Ce, TW):
        wd = min(TW, Ce - lo)
        blocks.append(
            Xe[:, lo:lo + wd].reshape(nk // 2, 2, 128, wd).transpose(2, 0, 1, 3)
            .reshape(128, nk * wd)
        )
    return np.ascontiguousarray(np.concatenate(blocks, axis=1))


def _prep_inputs(inputs):
    f32 = np.float32
    x, percore, C = _route(inputs)

    w13 = np.array(inputs["e3_W1"], f32, copy=True)
    w13[: 3 * D] *= f32(np.asarray(inputs["e3_a"]).reshape(-1)[0])
    w13[3 * D:] *= f32(np.asarray(inputs["e3_b"]).reshape(-1)[0])
    w1s = [np.asarray(inputs["e0_W1"], f32), np.asarray(inputs["e1_W1"], f32),
           np.asarray(inputs["e2_W1"], f32), w13]

    common = {f"w1_{e}": _pair_pack(w1s[e], W1S) for e in range(NE)}
    common["w2all"] = np.concatenate(
        [_pair_pack(np.asarray(inputs[f"e{e}_W2"], f32), W2S) for e in range(NE)],
        axis=1,
    )
    common["dw1"] = _pair_pack(np.asarray(inputs["dec_W1"], f32), DW1S)
    dw2 = np.asarray(inputs["dec_W2"], f32)
    common["dw2"] = np.ascontiguousarray(
        (dw2 * DW2S).reshape(2, 2, 128, NCLS).transpose(2, 0, 1, 3)
        .reshape(128, 4 * NCLS).astype(F8NP)
    )
    b1p = np.concatenate(
        [np.asarray(inputs[f"e{e}_b1"], f32) * f32(W1S) for e in range(NE)]
    ).reshape(16, 128).T
    b2p = np.concatenate(
        [np.asarray(inputs[f"e{e}_b2"], f32) for e in range(NE)]
    ).reshape(16, 128).T
    db1p = (np.asarray(inputs["dec_b1"], f32) * 0.5).reshape(4, 128).T
    common["biasp"] = np.ascontiguousarray(
        np.concatenate([b1p, b2p, db1p], axis=1)
    )

    in_maps = []
    for c in range(NCORES):
        m = dict(common)
        for e in range(NE):
            if C[e]:
                m[f"x{e}"] = _pack_x(x, percore[e][c], e, C[e])
        in_maps.append(m)
    return in_maps, percore, C


def kernel(**inputs):
    in_maps, percore, C = _prep_inputs(inputs)
    nc = _get_nc(C)
    res = run_bass_kernel_spmd(nc, in_maps, core_ids=list(range(NCORES)))
    offs = np.concatenate([[0], np.cumsum(C)])
    out = np.empty((B, NCLS), np.float32)
    for c in range(NCORES):
        r = res.results[c]["out"]
        for e in range(NE):
            ic = percore[e][c]
            if len(ic):
                out[ic] = r[offs[e]:offs[e] + len(ic)]
    # dec2 ran on tanh/2 - 0.5 centered activations; fold the 0.5*colsum(W2)
    # constant and dec_b2 back in here.
    dw2 = np.asarray(inputs["dec_W2"], np.float64)
    out += (np.asarray(inputs["dec_b2"], np.float64).reshape(1, NCLS)
            + 0.5 * dw2.sum(axis=0).reshape(1, NCLS)).astype(np.float32)
    return out
